# revision 3
# baseline (speedup 1.0000x reference)
"""Trainium2 Bass kernel for nn_DiffusionCNN — v2 (window-gather design).

Why v2: profiling showed the baseline spends 87% of its time on the GPSIMD
(Pool) engine generating one DMA descriptor per gathered 256B row (27 rows
per output voxel per conv).  v2 cuts descriptors 3x and makes each one a
contiguous 768B window:

  - Tables (x and h1) are stored in "zero-padded run" form: voxels in row
    order, with 0/1/2 zero rows inserted between consecutive voxels so that
    for EVERY queried 3x1x1 (dz) neighbor triple there is a table position w
    with rows [w, w+1, w+2] = [val(key-1), val(key), val(key+1)] (zeros for
    missing voxels).  All structure is derived from neighbor_idx alone.
  - A 3x3x3 conv then needs only 9 window gathers per output voxel (one per
    (dx,dy) group); each window is one 768B descriptor, gathered with the
    SWDGE dma_gather in transpose mode (elem_size=384, elem_step=128) so
    data lands channel-major and feeds the PE directly:
       out_tile = sum_{g,s} W[g,s]^T @ win_g[:, s, :]
  - Tables are tiled into 1536-position blocks aligned with 512-slot output
    tiles (block breaks only at run boundaries), so per-tile gather index
    bases are the same for all 8 cores (int16 local indices, shared SPMD
    program).
  - Phase 1 (conv1) writes h1 rows into the padded h1 table with per-128-row
    indirect DMA scatters; phase 2 (conv2 + pointwise MLP) mirrors the
    baseline tail.
  - Gathers rotate over 4 SWDGE queues.

Host-side work: sharding, padded-table/position construction, window index
tables, and re-assembly of the output (pure index marshalling).
"""

import numpy as np
import ml_dtypes

# ---------------------------------------------------------------- constants
N = 200000
PER = 25000
NCORES = 8
C = 128
K = 27
TEMB = 6
IN_CH = 7

SL = 512                  # slots per tile
BLK = 1536                # table positions per block
HALO = 2048               # slots of halo on each side, conv-range to conv-range
NT1 = 58                  # conv1 tiles (x-blocks 4..61)
NT2 = 50                  # conv2 tiles (= conv1 tiles 4..53)
XBLOCKS = 66
NXSLOTS = XBLOCKS * SL    # 33792 virtual x-range slots
XP = 104448               # x table positions (>= 1536*57 + 16386)
HP = 122880               # h1 table positions (>= 1536*57 + 32768, 30*4096)
DUMPL = 1532              # local scatter dump: block tail slot never read
                          # (fill <= 1530 -> reads <= 1531; zero-window 1533+)
WIN = 16384               # gather in_ap row span per tile
KI = 9 * SL               # window indices per gather instruction (4608)
BLKFILL_MAX = 1530        # max used positions per block (zero-window at +1533)

_bf16 = ml_dtypes.bfloat16

K_OF = {}
_k = 0
for _dx in (-1, 0, 1):
    for _dy in (-1, 0, 1):
        for _dz in (-1, 0, 1):
            K_OF[(_dx, _dy, _dz)] = _k
            _k += 1
GROUPS = [(dx, dy) for dx in (-1, 0, 1) for dy in (-1, 0, 1)]


# ------------------------------------------------------------- device program
def _build_program(bench_reps=0):
    import concourse.bass as bass
    import concourse.mybir as mybir
    import concourse.tile as tile
    from concourse import bacc

    bf = mybir.dt.bfloat16
    f32 = mybir.dt.float32
    i16 = mybir.dt.int16
    i32 = mybir.dt.int32
    AF = mybir.ActivationFunctionType

    nc = bacc.Bacc("TRN2", target_bir_lowering=False, debug=False,
                   num_swdge_queues=1, dynamic_dma_scratch_size=32768)

    x_tab = nc.dram_tensor("x_tab", [XP, C], bf, kind="ExternalInput")
    i1 = nc.dram_tensor("i1", [128, NT1 * KI // 16], i16, kind="ExternalInput")
    i2 = nc.dram_tensor("i2", [128, NT2 * KI // 16], i16, kind="ExternalInput")
    sc = nc.dram_tensor("sc", [128, NT1 * SL // 16], i16, kind="ExternalInput")
    w1 = nc.dram_tensor("w1", [C, K * C], bf, kind="ExternalInput")
    w2 = nc.dram_tensor("w2", [C, K * C], bf, kind="ExternalInput")
    w3 = nc.dram_tensor("w3", [C, C], bf, kind="ExternalInput")
    w4 = nc.dram_tensor("w4", [C, 16], bf, kind="ExternalInput")
    b1 = nc.dram_tensor("b1", [C, 1], f32, kind="ExternalInput")
    b2 = nc.dram_tensor("b2", [C, 1], f32, kind="ExternalInput")
    b3 = nc.dram_tensor("b3", [C, 1], f32, kind="ExternalInput")
    b4 = nc.dram_tensor("b4", [1, 1], f32, kind="ExternalInput")
    outd = nc.dram_tensor("out", [NT2 * SL], f32, kind="ExternalOutput")
    h1tab = nc.dram_tensor("h1_tab", [HP, C], bf, kind="Internal")

    with tile.TileContext(nc) as tc:
        with (
            tc.tile_pool(name="const", bufs=1) as constp,
            tc.tile_pool(name="idx", bufs=3) as idxp,
            tc.tile_pool(name="gat", bufs=2) as gatp,
            tc.tile_pool(name="act", bufs=3) as actp,
            tc.tile_pool(name="stage", bufs=2) as stagep,
            tc.tile_pool(name="psacc", bufs=2, space="PSUM") as psacc,
            tc.tile_pool(name="pstr", bufs=2, space="PSUM") as pstr,
            tc.tile_pool(name="psout", bufs=2, space="PSUM") as psout,
        ):
            w1_sb = constp.tile([C, K * C], bf, tag="w1")
            nc.sync.dma_start(w1_sb[:], w1[:])
            w2_sb = constp.tile([C, K * C], bf, tag="w2")
            nc.sync.dma_start(w2_sb[:], w2[:])
            w3_sb = constp.tile([C, C], bf, tag="w3")
            nc.sync.dma_start(w3_sb[:], w3[:])
            w4_sb = constp.tile([C, 16], bf, tag="w4")
            nc.sync.dma_start(w4_sb[:], w4[:])
            b1_sb = constp.tile([C, 1], f32, tag="b1")
            nc.sync.dma_start(b1_sb[:], b1[:])
            b2_sb = constp.tile([C, 1], f32, tag="b2")
            nc.sync.dma_start(b2_sb[:], b2[:])
            b3_sb = constp.tile([C, 1], f32, tag="b3")
            nc.sync.dma_start(b3_sb[:], b3[:])
            b4_sb = constp.tile([1, 1], f32, tag="b4")
            nc.sync.dma_start(b4_sb[:], b4[:])
            from concourse.masks import make_identity
            ident = constp.tile([C, C], bf, tag="ident")
            make_identity(nc, ident[:])
            zsb = constp.tile([128, 4096], bf, tag="zsb")
            nc.vector.memset(zsb[:], 0.0)

            def zero_h1tab():
                # HP rows * 128ch zeroed in chunks of [128, 4096]
                nch = HP // 4096
                for j in range(nch):
                    nc.sync.dma_start(
                        h1tab[j * 4096:(j + 1) * 4096, :].rearrange(
                            "(p a) e -> p (a e)", p=128),
                        zsb[:],
                    )
                # Pool-engine read touching every zeroed chunk: the tile
                # framework inserts waits for all 30 zero-DMA completions,
                # and Pool is in-order, so all later scatters/gathers are
                # safely after the zeroing.
                zchk = idxp.tile([30, 64], bf, tag="zchk")
                nc.gpsimd.dma_start(
                    zchk[:],
                    bass.AP(h1tab, 0, [[4096 * C, 30], [1, 64]]),
                )

            def win_gather(tab, idx_dram, t, q):
                it = idxp.tile([128, KI // 16], i16, tag="it")
                nc.sync.dma_start(
                    it[:], idx_dram[:, t * (KI // 16):(t + 1) * (KI // 16)]
                )
                g = gatp.tile([128, 3 * KI], bf, tag="g")
                in_ap = bass.AP(tab, 1536 * t * C, [[C, WIN], [1, 384]])
                nc.gpsimd.dma_gather(
                    out_ap=g[:].rearrange("p (m e) -> p m e", e=384),
                    in_ap=in_ap,
                    idxs_ap=it[:, :],
                    num_idxs=KI,
                    num_idxs_reg=KI,
                    elem_size=384,
                    elem_step=C,
                    transpose=False,
                    single_packet=False,
                    queue_num=0,
                )
                return g

            def conv_acc(g, w_sb):
                # g: [128, 36, 384] row-major windows (window n at partition
                # n%128, chunk n//128).  For each (grp, s): transpose the 4
                # chunk-slices [128w, 128ch] -> [128ch, 128w] into one PSUM
                # tile, copy to SBUF, matmul-accumulate.
                ps = psacc.tile([C, SL], f32, tag="acc")
                for k in range(K):
                    grp, s = divmod(k, 3)
                    pt2 = pstr.tile([C, SL], bf, tag="tr2")
                    for c4 in range(4):
                        m = 4 * grp + c4
                        nc.tensor.matmul(
                            pt2[:, 128 * c4:128 * (c4 + 1)],
                            lhsT=g[:, m * 384 + s * 128:m * 384 + (s + 1) * 128],
                            rhs=ident[:],
                            is_transpose=True,
                            start=(c4 == 0),
                            stop=(c4 == 3),
                        )
                    ts = actp.tile([C, SL], bf, tag="ts")
                    if k % 2 == 0:
                        nc.vector.tensor_copy(ts[:], pt2[:])
                    else:
                        nc.scalar.copy(ts[:], pt2[:])
                    nc.tensor.matmul(
                        ps[:],
                        lhsT=w_sb[:, C * k:C * (k + 1)],
                        rhs=ts[:],
                        start=(k == 0),
                        stop=(k == K - 1),
                    )
                return ps

            wup = constp.tile([128, 3 * 128], bf, tag="wup")
            wui = constp.tile([128, 8], i16, tag="wui")
            nc.vector.memset(wui[:], 0)

            def warmup_queues():
                for q in range(1):
                    for _ in range(2):
                        nc.gpsimd.dma_gather(
                            out_ap=wup[:].rearrange("p (m e) -> p m e", e=384),
                            in_ap=bass.AP(x_tab, 0, [[C, WIN], [1, 384]]),
                            idxs_ap=wui[:, :],
                            num_idxs=128,
                            num_idxs_reg=128,
                            elem_size=384,
                            elem_step=C,
                            transpose=False,
                            single_packet=False,
                            queue_num=0,
                        )

            _first = [True]

            def emit_body():
                if _first[0]:
                    warmup_queues()
                    _first[0] = False
                zero_h1tab()
                # ---------------- phase 1: conv1 -> h1 table ----------------
                for t in range(NT1):
                    g = win_gather(x_tab, i1, t, t % 4)
                    ps = conv_acc(g, w1_sb)
                    h1T = actp.tile([C, SL], bf, tag="h")
                    nc.scalar.activation(h1T[:], ps[:], AF.Silu,
                                         bias=b1_sb[:, 0:1])
                    pt = pstr.tile([C, SL], bf, tag="tr")
                    for cch in range(4):
                        nc.tensor.matmul(
                            pt[:, 128 * cch:128 * (cch + 1)],
                            lhsT=h1T[:, 128 * cch:128 * (cch + 1)],
                            rhs=ident[:],
                            is_transpose=True,
                            start=(cch == 0),
                            stop=(cch == 3),
                        )
                    st = stagep.tile([C, SL], bf, tag="st")
                    nc.vector.tensor_copy(st[:], pt[:])
                    sct = idxp.tile([128, SL // 16], i16, tag="sct")
                    nc.sync.dma_start(
                        sct[:], sc[:, t * (SL // 16):(t + 1) * (SL // 16)])
                    nc.gpsimd.dma_scatter_add(
                        out_ap=h1tab[1536 * t:1536 * t + 32768, :],
                        in_ap=st[:].rearrange("p (c e) -> p c e", e=C),
                        idxs_ap=sct[:, :],
                        num_idxs=SL,
                        num_idxs_reg=SL,
                        elem_size=C,
                        single_packet=False,
                        queue_num=0,
                    )

                nc.gpsimd.drain()
                # ---------------- phase 2: conv2 + MLP ----------------------
                for t in range(NT2):
                    g = win_gather(h1tab, i2, t, t % 4)
                    ps = conv_acc(g, w2_sb)
                    h2 = actp.tile([C, SL], bf, tag="h")
                    nc.scalar.activation(h2[:], ps[:], AF.Silu,
                                         bias=b2_sb[:, 0:1])
                    ps3 = psacc.tile([C, SL], f32, tag="acc")
                    nc.tensor.matmul(ps3[:], lhsT=w3_sb[:], rhs=h2[:],
                                     start=True, stop=True)
                    h3 = actp.tile([C, SL], bf, tag="h")
                    nc.scalar.activation(h3[:], ps3[:], AF.Silu,
                                         bias=b3_sb[:, 0:1])
                    ps4 = psout.tile([1, SL], f32, tag="o")
                    nc.tensor.matmul(ps4[:], lhsT=w4_sb[:, 0:1], rhs=h3[:],
                                     start=True, stop=True)
                    ot = stagep.tile([1, SL], f32, tag="ot")
                    nc.scalar.activation(ot[0:1, :], ps4[:], AF.Identity,
                                         bias=b4_sb[0:1, 0:1])
                    nc.sync.dma_start(outd[None, t * SL:(t + 1) * SL],
                                      ot[0:1, :])
                nc.gpsimd.drain()

            if bench_reps > 0:
                with tc.For_i(0, bench_reps, 1):
                    emit_body()
            else:
                emit_body()

    nc.compile()
    return nc


_NC_CACHE = {}


def _get_nc():
    if "nc" not in _NC_CACHE:
        _NC_CACHE["nc"] = _build_program()
    return _NC_CACHE["nc"]


# ------------------------------------------------------------------ host prep
def _sinusoidal(t):
    half = TEMB // 2
    freqs = (np.float32(2.0) ** np.arange(half, dtype=np.float32)) \
        * np.float32(np.pi)
    ang = t.astype(np.float32)[:, None] * freqs[None, :]
    return np.concatenate([np.sin(ang), np.cos(ang)], -1).astype(np.float32)


def _wrap_idx(flat):
    """[T, KI] int -> [128, T*KI/16] int16 SWDGE layout (16-wrap, x8)."""
    T = flat.shape[0]
    a = flat.reshape(T, KI // 16, 16).transpose(2, 0, 1).reshape(
        16, T * (KI // 16))
    return np.tile(a, (8, 1)).astype(np.int16)


def _wrap_scatter(scpos):
    """[T, SL] -> [128, T*SL/16] int16 (16-wrap, x8) like gather indices."""
    T = scpos.shape[0]
    a = scpos.reshape(T, SL // 16, 16).transpose(2, 0, 1).reshape(
        16, T * (SL // 16))
    return np.tile(a, (8, 1)).astype(np.int16)


def _global_flags(nidx):
    """cont[r]: row r+1 continues r's z-run; shared[r]: rows r,r+1 straddle a
    single queried missing address (1 zero between them)."""
    cont = np.zeros(N, bool)
    k14 = K_OF[(0, 0, 1)]
    nxt = nidx[k14, :]
    rr = np.arange(N)
    cont[:-1] = nxt[:-1] == rr[1:]
    shared = np.zeros(N, bool)
    for dx, dy in GROUPS:
        km = K_OF[(dx, dy, -1)]
        k0 = K_OF[(dx, dy, 0)]
        kp = K_OF[(dx, dy, 1)]
        m = (nidx[k0] == N) & (nidx[km] < N) & (nidx[kp] < N)
        if m.any():
            jm = nidx[km, m].astype(np.int64)
            jp = nidx[kp, m].astype(np.int64)
            assert np.all(jp == jm + 1), "shared-pair rows not adjacent"
            shared[jm] = True
    return cont, shared


def _build_core(core, nidx, cont, shared, x_full):
    s = core * PER
    x0 = s - 2 * HALO  # global row of x-slot 0

    # ---- breaks: c[b] for b in 0..XBLOCKS, on slot indices ----
    # valid break between slots j-1, j  <=>  rows r-1, r not run-continuing
    # and not a shared pair (absent slots always valid).
    def brk_valid(j):
        r = x0 + j
        if r <= 0 or r >= N:
            return True
        return not (cont[r - 1] or shared[r - 1])

    c = np.zeros(XBLOCKS + 1, np.int64)
    for b in range(1, XBLOCKS + 1):
        j = min(c[b - 1] + SL, NXSLOTS)
        if b < XBLOCKS:
            # rows past slot NXSLOTS are never queried; the final break may
            # land mid-run harmlessly
            while j > c[b - 1] and not brk_valid(j):
                j -= 1
        assert j > c[b - 1], f"no valid break in block {b}"
        assert j > b * SL - 64, f"break drifted too far: block {b} at {j}"
        c[b] = j
    assert c[XBLOCKS] >= NXSLOTS - 64

    # ---- positions for x table (blocks 0..65) and h1 table (conv1 tiles,
    #      i.e. x-blocks 4..61 -> h1 blocks 0..57) ----
    def build_positions(nblocks, block_of_slot_base):
        """pos[slot] for present slots, per-block padded-run layout."""
        pos = np.full(NXSLOTS, -1, np.int64)
        for b in range(nblocks):
            lo, hi = c[block_of_slot_base + b], c[block_of_slot_base + b + 1]
            p = b * BLK + 2
            prev_r = None
            for j in range(lo, hi):
                r = x0 + j
                if not (0 <= r < N):
                    continue
                if prev_r is not None:
                    if cont[prev_r] and prev_r == r - 1:
                        pass
                    elif shared[prev_r] and prev_r == r - 1:
                        p += 1
                    else:
                        p += 2
                pos[j] = p
                p += 1
                prev_r = r
            assert p - b * BLK <= BLKFILL_MAX, (core, b, p - b * BLK)
        return pos

    # x table positions over x-blocks 0..65 (block b at positions 1536b)
    pos_x = build_positions(XBLOCKS, 0)
    # h1 table positions over conv1 tiles 0..57 (= x-blocks 4..61)
    pos_h = build_positions(NT1, 4)

    # map global row -> slot (rows outside x range -> -1)
    def row_slot(r):
        j = r - x0
        return j

    # ---- x table content ----
    x_tab = np.zeros((XP, C), np.float32)
    jj = np.arange(NXSLOTS)
    rr = x0 + jj
    pres = (rr >= 0) & (rr < N) & (pos_x >= 0)
    x_tab[pos_x[pres], :IN_CH] = x_full[rr[pres]]
    x_tab = x_tab.astype(_bf16)

    # ---- window index builder ----
    def windows(tile_blocks_base, ntiles, pos, pos_base_blockoff):
        """For tiles t=0..ntiles-1 over slots [c[B+t], c[B+t+1]),
        B = tile_blocks_base: per group windows, int16 local vs 1536*t.

        pos: position array indexed by slot; pos_base_blockoff: x-block index
        whose positions correspond to pos block 0 (4 for h1 table).
        Returns idx [ntiles, KI] int64 and (slots list per tile)."""
        idx = np.zeros((ntiles, KI), np.int64)
        tile_rows = []
        for t in range(ntiles):
            B = tile_blocks_base + t
            lo, hi = c[B], c[B + 1]
            base = 1536 * (B - pos_base_blockoff - 4)
            zero_w = base + 1533
            rows_here = np.full(SL, -1, np.int64)
            for gi, (dx, dy) in enumerate(GROUPS):
                km = K_OF[(dx, dy, -1)]
                k0 = K_OF[(dx, dy, 0)]
                kp = K_OF[(dx, dy, 1)]
                for u in range(SL):
                    j = lo + u
                    w = zero_w
                    if j < hi:
                        r = x0 + j
                        if 0 <= r < N:
                            rows_here[u] = r
                            j0 = nidx[k0, r]
                            if j0 < N:
                                w = pos[row_slot(j0)] - 1
                            else:
                                jm = nidx[km, r]
                                jp = nidx[kp, r]
                                if jm < N:
                                    w = pos[row_slot(jm)]
                                elif jp < N:
                                    w = pos[row_slot(jp)] - 2
                    lw = w - base
                    assert 0 <= lw <= WIN - 3, (core, t, gi, u, lw)
                    idx[t, gi * SL + u] = lw
            tile_rows.append(rows_here)
        return idx, tile_rows

    # conv1: tiles over x-blocks 4..61, windows into x table (x-block b sits
    # at position 1536b; base must be 1536*t = 1536*(B-4))
    idx1, rows1 = windows(4, NT1, pos_x, 0)
    # conv2: tiles over x-blocks 8..57 (= conv1 tiles 4..53), windows into h1
    # table where h1 block 0 = x-block 4 (so position of block B is
    # 1536*(B-4)); need base = 1536*t = 1536*(B-8) -> pos_base_blockoff = 4.
    idx2, rows2 = windows(8, NT2, pos_h, 4)

    # ---- scatter positions for conv1 tiles (local to slice 1536t) ----
    scpos = np.full((NT1, SL), DUMPL, np.int64)
    for t in range(NT1):
        rh = rows1[t]
        for u in range(SL):
            r = rh[u]
            if r >= 0:
                p = pos_h[row_slot(r)]
                assert p >= 0
                lp = p - 1536 * t
                assert 0 <= lp < 32000, (core, t, u, lp)
                scpos[t, u] = lp
    scw = _wrap_scatter(scpos)

    # ---- output mapping ----
    outmap = []  # (global_row, t2*SL + u)
    e = s + PER
    for t in range(NT2):
        rh = rows2[t]
        for u in range(SL):
            r = rh[u]
            if s <= r < e:
                outmap.append((r, t * SL + u))
    assert len(outmap) == PER, (core, len(outmap))

    return {
        "x_tab": x_tab,
        "i1": _wrap_idx(idx1),
        "i2": _wrap_idx(idx2),
        "sc": scw,
    }, outmap


def _prep_shared(W1, b1, W2, b2, W3, b3, W4, b4):
    W1p = np.zeros((K, C, C), np.float32)
    W1p[:, :IN_CH, :] = W1
    w1d = np.ascontiguousarray(
        W1p.transpose(1, 0, 2).reshape(C, K * C)).astype(_bf16)
    w2d = np.ascontiguousarray(
        W2.transpose(1, 0, 2).reshape(C, K * C)).astype(_bf16)
    w3d = np.ascontiguousarray(W3).astype(_bf16)
    w4d = np.zeros((C, 16), _bf16)
    w4d[:, 0] = W4[:, 0].astype(_bf16)
    return {
        "w1": w1d, "w2": w2d, "w3": w3d, "w4": w4d,
        "b1": np.ascontiguousarray(b1.reshape(C, 1), dtype=np.float32),
        "b2": np.ascontiguousarray(b2.reshape(C, 1), dtype=np.float32),
        "b3": np.ascontiguousarray(b3.reshape(C, 1), dtype=np.float32),
        "b4": np.ascontiguousarray(b4.reshape(1, 1), dtype=np.float32),
    }


_PREP_CACHE = {}


def _prep_in_maps(inputs):
    key = id(inputs.get("neighbor_idx"))
    nidx = np.asarray(inputs["neighbor_idx"]).astype(np.int32)
    features = np.asarray(inputs["features"], np.float32)
    t = np.asarray(inputs["t"])
    x_full = np.concatenate([features, _sinusoidal(t)], -1)
    shared_w = _prep_shared(
        np.asarray(inputs["W1"], np.float32), np.asarray(inputs["b1"], np.float32),
        np.asarray(inputs["W2"], np.float32), np.asarray(inputs["b2"], np.float32),
        np.asarray(inputs["W3"], np.float32), np.asarray(inputs["b3"], np.float32),
        np.asarray(inputs["W4"], np.float32), np.asarray(inputs["b4"], np.float32),
    )
    cont, shp = _global_flags(nidx)
    in_maps, outmaps = [], []
    for core in range(NCORES):
        m, om = _build_core(core, nidx, cont, shp, x_full)
        m.update(shared_w)
        in_maps.append(m)
        outmaps.append(om)
    return in_maps, outmaps


# ------------------------------------------------------------------ execution
def _run_pjrt(nc, in_maps, reps=0):
    import time as _time
    import jax
    from jax.sharding import Mesh, NamedSharding, PartitionSpec
    from jax.experimental.shard_map import shard_map
    import concourse.mybir as mybir
    from concourse import bass2jax

    bass2jax.install_neuronx_cc_hook()

    n_cores = len(in_maps)
    partition_name = (
        nc.partition_id_tensor.name if nc.partition_id_tensor else None
    )
    in_names, out_names, out_avals, zero_outs = [], [], [], []
    for alloc in nc.m.functions[0].allocations:
        if not isinstance(alloc, mybir.MemoryLocationSet):
            continue
        name = alloc.memorylocations[0].name
        if alloc.kind == "ExternalInput":
            if name != partition_name:
                in_names.append(name)
        elif alloc.kind == "ExternalOutput":
            shape = tuple(alloc.tensor_shape)
            dtype = mybir.dt.np(alloc.dtype)
            out_names.append(name)
            out_avals.append(jax.core.ShapedArray(shape, dtype))
            zero_outs.append(np.zeros(shape, dtype))
    n_params = len(in_names)
    n_outs = len(out_names)
    all_names = in_names + out_names
    if partition_name is not None:
        all_names = all_names + [partition_name]
    donate = tuple(range(n_params, n_params + n_outs))

    def _body(*args):
        operands = list(args)
        if partition_name is not None:
            operands.append(bass2jax.partition_id_tensor())
        outs = bass2jax._bass_exec_p.bind(
            *operands,
            out_avals=tuple(out_avals),
            in_names=tuple(all_names),
            out_names=tuple(out_names),
            lowering_input_output_aliases=(),
            sim_require_finite=True,
            sim_require_nnan=True,
            nc=nc,
        )
        return tuple(outs)

    devices = jax.devices()[:n_cores]
    mesh = Mesh(np.asarray(devices), ("core",))
    spec = PartitionSpec("core")
    sharded = jax.jit(
        shard_map(_body, mesh=mesh, in_specs=(spec,) * (n_params + n_outs),
                  out_specs=(spec,) * n_outs, check_rep=False),
        donate_argnums=donate,
        keep_unused=True,
    )
    concat_in = [
        np.concatenate([np.asarray(m[name]) for m in in_maps], axis=0)
        for name in in_names
    ]
    sh = NamedSharding(mesh, spec)
    inp_dev = [jax.device_put(a, sh) for a in concat_in]

    def _zeros():
        return [np.zeros((n_cores * z.shape[0], *z.shape[1:]), z.dtype)
                for z in zero_outs]

    out_arrs = sharded(*inp_dev, *_zeros())
    jax.block_until_ready(out_arrs)
    results = [
        {name: np.asarray(out_arrs[i]).reshape(n_cores, *out_avals[i].shape)[c]
         for i, name in enumerate(out_names)}
        for c in range(n_cores)
    ]

    times = []
    for _ in range(reps):
        zs = _zeros()
        t0 = _time.perf_counter()
        o = sharded(*inp_dev, *zs)
        jax.block_until_ready(o)
        times.append(_time.perf_counter() - t0)
    return results, times


def _assemble(results, outmaps):
    out = np.empty((N, 1), np.float32)
    for core in range(NCORES):
        r = results[core]["out"]
        for row, slot in outmaps[core]:
            out[row, 0] = r[slot]
    return out


def kernel(**inputs) -> np.ndarray:
    in_maps, outmaps = _prep_in_maps(inputs)
    nc = _get_nc()
    results, _ = _run_pjrt(nc, in_maps, reps=0)
    return _assemble(results, outmaps)


def bench(inputs, loop_reps=(1, 26), wall_reps=8):
    in_maps, outmaps = _prep_in_maps(inputs)
    walls = {}
    outs = {}
    for R in loop_reps:
        nc = _build_program(bench_reps=R)
        results, times = _run_pjrt(nc, in_maps, reps=wall_reps)
        walls[R] = min(times)
        outs[R] = _assemble(results, outmaps)
    R1, R2 = loop_reps
    per_iter = (walls[R2] - walls[R1]) / (R2 - R1)
    return per_iter, walls, outs


# revision 4
# speedup vs baseline: 1.4666x; 1.4666x over previous
"""Trainium2 Bass kernel for nn_DiffusionCNN — v2 (window-gather design).

Why v2: profiling showed the baseline spends 87% of its time on the GPSIMD
(Pool) engine generating one DMA descriptor per gathered 256B row (27 rows
per output voxel per conv).  v2 cuts descriptors 3x and makes each one a
contiguous 768B window:

  - Tables (x and h1) are stored in "zero-padded run" form: voxels in row
    order, with 0/1/2 zero rows inserted between consecutive voxels so that
    for EVERY queried 3x1x1 (dz) neighbor triple there is a table position w
    with rows [w, w+1, w+2] = [val(key-1), val(key), val(key+1)] (zeros for
    missing voxels).  All structure is derived from neighbor_idx alone.
  - A 3x3x3 conv then needs only 9 window gathers per output voxel (one per
    (dx,dy) group); each window is one 768B read (elem_size=384,
    elem_step=128), gathered with SWDGE dma_gather in NON-transpose mode
    (2 descriptors/window vs 4 in transpose mode - descriptor generation on
    the single Q7 SWDGE queue is the kernel's bottleneck).  Windows land
    row-major and are transposed to channel-major on the PE (4 is_transpose
    matmuls + 1 PSUM->SBUF copy per (group, dz)), then
       out_tile = sum_{g,s} W[g,s]^T @ win_g[:, s, :]
  - Tables are tiled into 1536-position blocks aligned with 512-slot output
    tiles (block breaks only at run boundaries), so per-tile gather index
    bases are the same for all 8 cores (int16 local indices, shared SPMD
    program).
  - Phase 1 (conv1) writes h1 rows into the padded h1 table with per-128-row
    indirect DMA scatters; phase 2 (conv2 + pointwise MLP) mirrors the
    baseline tail.
  - Everything stays on SWDGE queue 0: multi-queue SWDGE (num_swdge_queues
    >= 2) corrupts concurrently in-flight gathers on this stack.

Host-side work: sharding, padded-table/position construction, window index
tables, and re-assembly of the output (pure index marshalling).
"""

import numpy as np
import ml_dtypes

# ---------------------------------------------------------------- constants
N = 200000
PER = 25000
NCORES = 8
C = 128
K = 27
TEMB = 6
IN_CH = 7

SL = 512                  # slots per tile
BLK = 1536                # table positions per block
HALO = 2048               # slots of halo on each side, conv-range to conv-range
NT1 = 58                  # conv1 tiles (x-blocks 4..61)
NT2 = 50                  # conv2 tiles (= conv1 tiles 4..53)
XBLOCKS = 66
NXSLOTS = XBLOCKS * SL    # 33792 virtual x-range slots
XP = 104448               # x table positions (>= 1536*57 + 16386)
HP = 122880               # h1 table positions (>= 1536*57 + 32768, 30*4096)
DUMPL = 1532              # local scatter dump: block tail slot never read
                          # (fill <= 1530 -> reads <= 1531; zero-window 1533+)
WIN = 16384               # gather in_ap row span per tile
KI = 9 * SL               # window indices per gather instruction (4608)
BLKFILL_MAX = 1530        # max used positions per block (zero-window at +1533)

_bf16 = ml_dtypes.bfloat16

K_OF = {}
_k = 0
for _dx in (-1, 0, 1):
    for _dy in (-1, 0, 1):
        for _dz in (-1, 0, 1):
            K_OF[(_dx, _dy, _dz)] = _k
            _k += 1
GROUPS = [(dx, dy) for dx in (-1, 0, 1) for dy in (-1, 0, 1)]


# ------------------------------------------------------------- device program
def _build_program(bench_reps=0):
    import concourse.bass as bass
    import concourse.mybir as mybir
    import concourse.tile as tile
    from concourse import bacc

    bf = mybir.dt.bfloat16
    f32 = mybir.dt.float32
    i16 = mybir.dt.int16
    i32 = mybir.dt.int32
    AF = mybir.ActivationFunctionType

    nc = bacc.Bacc("TRN2", target_bir_lowering=False, debug=False,
                   num_swdge_queues=1, dynamic_dma_scratch_size=32768)

    x_tab = nc.dram_tensor("x_tab", [XP, C], bf, kind="ExternalInput")
    i1 = nc.dram_tensor("i1", [128, NT1 * KI // 16], i16, kind="ExternalInput")
    i2 = nc.dram_tensor("i2", [128, NT2 * KI // 16], i16, kind="ExternalInput")
    sc = nc.dram_tensor("sc", [128, NT1 * SL // 16], i16, kind="ExternalInput")
    w1 = nc.dram_tensor("w1", [C, K * C], bf, kind="ExternalInput")
    w2 = nc.dram_tensor("w2", [C, K * C], bf, kind="ExternalInput")
    w3 = nc.dram_tensor("w3", [C, C], bf, kind="ExternalInput")
    w4 = nc.dram_tensor("w4", [C, 16], bf, kind="ExternalInput")
    b1 = nc.dram_tensor("b1", [C, 1], f32, kind="ExternalInput")
    b2 = nc.dram_tensor("b2", [C, 1], f32, kind="ExternalInput")
    b3 = nc.dram_tensor("b3", [C, 1], f32, kind="ExternalInput")
    b4 = nc.dram_tensor("b4", [1, 1], f32, kind="ExternalInput")
    outd = nc.dram_tensor("out", [NT2 * SL], f32, kind="ExternalOutput")
    h1tab = nc.dram_tensor("h1_tab", [HP, C], bf, kind="Internal")

    with tile.TileContext(nc) as tc:
        with (
            tc.tile_pool(name="const", bufs=1) as constp,
            tc.tile_pool(name="idx", bufs=3) as idxp,
            tc.tile_pool(name="gat", bufs=2) as gatp,
            tc.tile_pool(name="act", bufs=3) as actp,
            tc.tile_pool(name="stage", bufs=2) as stagep,
            tc.tile_pool(name="psacc", bufs=2, space="PSUM") as psacc,
            tc.tile_pool(name="pstr", bufs=2, space="PSUM") as pstr,
            tc.tile_pool(name="psout", bufs=2, space="PSUM") as psout,
        ):
            w1_sb = constp.tile([C, K * C], bf, tag="w1")
            nc.sync.dma_start(w1_sb[:], w1[:])
            w2_sb = constp.tile([C, K * C], bf, tag="w2")
            nc.sync.dma_start(w2_sb[:], w2[:])
            w3_sb = constp.tile([C, C], bf, tag="w3")
            nc.sync.dma_start(w3_sb[:], w3[:])
            w4_sb = constp.tile([C, 16], bf, tag="w4")
            nc.sync.dma_start(w4_sb[:], w4[:])
            b1_sb = constp.tile([C, 1], f32, tag="b1")
            nc.sync.dma_start(b1_sb[:], b1[:])
            b2_sb = constp.tile([C, 1], f32, tag="b2")
            nc.sync.dma_start(b2_sb[:], b2[:])
            b3_sb = constp.tile([C, 1], f32, tag="b3")
            nc.sync.dma_start(b3_sb[:], b3[:])
            b4_sb = constp.tile([1, 1], f32, tag="b4")
            nc.sync.dma_start(b4_sb[:], b4[:])
            from concourse.masks import make_identity
            ident = constp.tile([C, C], bf, tag="ident")
            make_identity(nc, ident[:])
            zsb = constp.tile([128, 4096], bf, tag="zsb")
            nc.vector.memset(zsb[:], 0.0)

            def zero_h1tab():
                # HP rows * 128ch zeroed in chunks of [128, 4096]
                nch = HP // 4096
                for j in range(nch):
                    nc.sync.dma_start(
                        h1tab[j * 4096:(j + 1) * 4096, :].rearrange(
                            "(p a) e -> p (a e)", p=128),
                        zsb[:],
                    )
                # Pool-engine read touching every zeroed chunk: the tile
                # framework inserts waits for all 30 zero-DMA completions,
                # and Pool is in-order, so all later scatters/gathers are
                # safely after the zeroing.
                zchk = idxp.tile([30, 64], bf, tag="zchk")
                nc.gpsimd.dma_start(
                    zchk[:],
                    bass.AP(h1tab, 0, [[4096 * C, 30], [1, 64]]),
                )

            def win_gather(tab, idx_dram, t, q):
                it = idxp.tile([128, KI // 16], i16, tag="it")
                nc.sync.dma_start(
                    it[:], idx_dram[:, t * (KI // 16):(t + 1) * (KI // 16)]
                )
                g = gatp.tile([128, 3 * KI], bf, tag="g")
                in_ap = bass.AP(tab, 1536 * t * C, [[C, WIN], [1, 384]])
                nc.gpsimd.dma_gather(
                    out_ap=g[:].rearrange("p (m e) -> p m e", e=384),
                    in_ap=in_ap,
                    idxs_ap=it[:, :],
                    num_idxs=KI,
                    num_idxs_reg=KI,
                    elem_size=384,
                    elem_step=C,
                    transpose=False,
                    single_packet=False,
                    queue_num=0,
                )
                return g

            def conv_acc(g, w_sb):
                # g: [128, 36, 384] row-major windows (window n at partition
                # n%128, chunk n//128).  For each (grp, s): transpose the 4
                # chunk-slices [128w, 128ch] -> [128ch, 128w] into one PSUM
                # tile, copy to SBUF, matmul-accumulate.
                ps = psacc.tile([C, SL], f32, tag="acc")
                for k in range(K):
                    grp, s = divmod(k, 3)
                    pt2 = pstr.tile([C, SL], bf, tag="tr2")
                    for c4 in range(4):
                        m = 4 * grp + c4
                        nc.tensor.matmul(
                            pt2[:, 128 * c4:128 * (c4 + 1)],
                            lhsT=g[:, m * 384 + s * 128:m * 384 + (s + 1) * 128],
                            rhs=ident[:],
                            is_transpose=True,
                            start=(c4 == 0),
                            stop=(c4 == 3),
                        )
                    ts = actp.tile([C, SL], bf, tag="ts")
                    if k % 2 == 0:
                        nc.vector.tensor_copy(ts[:], pt2[:])
                    else:
                        nc.scalar.copy(ts[:], pt2[:])
                    nc.tensor.matmul(
                        ps[:],
                        lhsT=w_sb[:, C * k:C * (k + 1)],
                        rhs=ts[:],
                        start=(k == 0),
                        stop=(k == K - 1),
                    )
                return ps

            wup = constp.tile([128, 3 * 128], bf, tag="wup")
            wui = constp.tile([128, 8], i16, tag="wui")
            nc.vector.memset(wui[:], 0)

            def warmup_queues():
                for q in range(1):
                    for _ in range(2):
                        nc.gpsimd.dma_gather(
                            out_ap=wup[:].rearrange("p (m e) -> p m e", e=384),
                            in_ap=bass.AP(x_tab, 0, [[C, WIN], [1, 384]]),
                            idxs_ap=wui[:, :],
                            num_idxs=128,
                            num_idxs_reg=128,
                            elem_size=384,
                            elem_step=C,
                            transpose=False,
                            single_packet=False,
                            queue_num=0,
                        )

            _first = [True]

            def emit_body():
                if _first[0]:
                    warmup_queues()
                    _first[0] = False
                zero_h1tab()
                # ---------------- phase 1: conv1 -> h1 table ----------------
                for t in range(NT1):
                    g = win_gather(x_tab, i1, t, t % 4)
                    ps = conv_acc(g, w1_sb)
                    h1T = actp.tile([C, SL], bf, tag="h")
                    nc.scalar.activation(h1T[:], ps[:], AF.Silu,
                                         bias=b1_sb[:, 0:1])
                    pt = pstr.tile([C, SL], bf, tag="tr")
                    for cch in range(4):
                        nc.tensor.matmul(
                            pt[:, 128 * cch:128 * (cch + 1)],
                            lhsT=h1T[:, 128 * cch:128 * (cch + 1)],
                            rhs=ident[:],
                            is_transpose=True,
                            start=(cch == 0),
                            stop=(cch == 3),
                        )
                    st = stagep.tile([C, SL], bf, tag="st")
                    nc.vector.tensor_copy(st[:], pt[:])
                    sct = idxp.tile([128, SL // 16], i16, tag="sct")
                    nc.sync.dma_start(
                        sct[:], sc[:, t * (SL // 16):(t + 1) * (SL // 16)])
                    nc.gpsimd.dma_scatter_add(
                        out_ap=h1tab[1536 * t:1536 * t + 32768, :],
                        in_ap=st[:].rearrange("p (c e) -> p c e", e=C),
                        idxs_ap=sct[:, :],
                        num_idxs=SL,
                        num_idxs_reg=SL,
                        elem_size=C,
                        single_packet=False,
                        queue_num=0,
                    )

                nc.gpsimd.drain()
                # ---------------- phase 2: conv2 + MLP ----------------------
                for t in range(NT2):
                    g = win_gather(h1tab, i2, t, t % 4)
                    ps = conv_acc(g, w2_sb)
                    h2 = actp.tile([C, SL], bf, tag="h")
                    nc.scalar.activation(h2[:], ps[:], AF.Silu,
                                         bias=b2_sb[:, 0:1])
                    ps3 = psacc.tile([C, SL], f32, tag="acc")
                    nc.tensor.matmul(ps3[:], lhsT=w3_sb[:], rhs=h2[:],
                                     start=True, stop=True)
                    h3 = actp.tile([C, SL], bf, tag="h")
                    nc.scalar.activation(h3[:], ps3[:], AF.Silu,
                                         bias=b3_sb[:, 0:1])
                    ps4 = psout.tile([1, SL], f32, tag="o")
                    nc.tensor.matmul(ps4[:], lhsT=w4_sb[:, 0:1], rhs=h3[:],
                                     start=True, stop=True)
                    ot = stagep.tile([1, SL], f32, tag="ot")
                    nc.scalar.activation(ot[0:1, :], ps4[:], AF.Identity,
                                         bias=b4_sb[0:1, 0:1])
                    nc.sync.dma_start(outd[None, t * SL:(t + 1) * SL],
                                      ot[0:1, :])
                nc.gpsimd.drain()

            if bench_reps > 0:
                with tc.For_i(0, bench_reps, 1):
                    emit_body()
            else:
                emit_body()

    nc.compile()
    return nc


_NC_CACHE = {}


def _get_nc():
    if "nc" not in _NC_CACHE:
        _NC_CACHE["nc"] = _build_program()
    return _NC_CACHE["nc"]


# ------------------------------------------------------------------ host prep
def _sinusoidal(t):
    half = TEMB // 2
    freqs = (np.float32(2.0) ** np.arange(half, dtype=np.float32)) \
        * np.float32(np.pi)
    ang = t.astype(np.float32)[:, None] * freqs[None, :]
    return np.concatenate([np.sin(ang), np.cos(ang)], -1).astype(np.float32)


def _wrap_idx(flat):
    """[T, KI] int -> [128, T*KI/16] int16 SWDGE layout (16-wrap, x8)."""
    T = flat.shape[0]
    a = flat.reshape(T, KI // 16, 16).transpose(2, 0, 1).reshape(
        16, T * (KI // 16))
    return np.tile(a, (8, 1)).astype(np.int16)


def _wrap_scatter(scpos):
    """[T, SL] -> [128, T*SL/16] int16 (16-wrap, x8) like gather indices."""
    T = scpos.shape[0]
    a = scpos.reshape(T, SL // 16, 16).transpose(2, 0, 1).reshape(
        16, T * (SL // 16))
    return np.tile(a, (8, 1)).astype(np.int16)


def _global_flags(nidx):
    """cont[r]: row r+1 continues r's z-run; shared[r]: rows r,r+1 straddle a
    single queried missing address (1 zero between them)."""
    cont = np.zeros(N, bool)
    k14 = K_OF[(0, 0, 1)]
    nxt = nidx[k14, :]
    rr = np.arange(N)
    cont[:-1] = nxt[:-1] == rr[1:]
    shared = np.zeros(N, bool)
    for dx, dy in GROUPS:
        km = K_OF[(dx, dy, -1)]
        k0 = K_OF[(dx, dy, 0)]
        kp = K_OF[(dx, dy, 1)]
        m = (nidx[k0] == N) & (nidx[km] < N) & (nidx[kp] < N)
        if m.any():
            jm = nidx[km, m].astype(np.int64)
            jp = nidx[kp, m].astype(np.int64)
            assert np.all(jp == jm + 1), "shared-pair rows not adjacent"
            shared[jm] = True
    return cont, shared


def _build_core(core, nidx, cont, shared, x_full):
    s = core * PER
    x0 = s - 2 * HALO  # global row of x-slot 0

    # ---- breaks: c[b] for b in 0..XBLOCKS, on slot indices ----
    # valid break between slots j-1, j  <=>  rows r-1, r not run-continuing
    # and not a shared pair (absent slots always valid).
    def brk_valid(j):
        r = x0 + j
        if r <= 0 or r >= N:
            return True
        return not (cont[r - 1] or shared[r - 1])

    c = np.zeros(XBLOCKS + 1, np.int64)
    for b in range(1, XBLOCKS + 1):
        j = min(c[b - 1] + SL, NXSLOTS)
        if b < XBLOCKS:
            # rows past slot NXSLOTS are never queried; the final break may
            # land mid-run harmlessly
            while j > c[b - 1] and not brk_valid(j):
                j -= 1
        assert j > c[b - 1], f"no valid break in block {b}"
        assert j > b * SL - 64, f"break drifted too far: block {b} at {j}"
        c[b] = j
    assert c[XBLOCKS] >= NXSLOTS - 64

    # ---- positions for x table (blocks 0..65) and h1 table (conv1 tiles,
    #      i.e. x-blocks 4..61 -> h1 blocks 0..57) ----
    def build_positions(nblocks, block_of_slot_base):
        """pos[slot] for present slots, per-block padded-run layout."""
        pos = np.full(NXSLOTS, -1, np.int64)
        for b in range(nblocks):
            lo, hi = c[block_of_slot_base + b], c[block_of_slot_base + b + 1]
            p = b * BLK + 2
            prev_r = None
            for j in range(lo, hi):
                r = x0 + j
                if not (0 <= r < N):
                    continue
                if prev_r is not None:
                    if cont[prev_r] and prev_r == r - 1:
                        pass
                    elif shared[prev_r] and prev_r == r - 1:
                        p += 1
                    else:
                        p += 2
                pos[j] = p
                p += 1
                prev_r = r
            assert p - b * BLK <= BLKFILL_MAX, (core, b, p - b * BLK)
        return pos

    # x table positions over x-blocks 0..65 (block b at positions 1536b)
    pos_x = build_positions(XBLOCKS, 0)
    # h1 table positions over conv1 tiles 0..57 (= x-blocks 4..61)
    pos_h = build_positions(NT1, 4)

    # map global row -> slot (rows outside x range -> -1)
    def row_slot(r):
        j = r - x0
        return j

    # ---- x table content ----
    x_tab = np.zeros((XP, C), np.float32)
    jj = np.arange(NXSLOTS)
    rr = x0 + jj
    pres = (rr >= 0) & (rr < N) & (pos_x >= 0)
    x_tab[pos_x[pres], :IN_CH] = x_full[rr[pres]]
    x_tab = x_tab.astype(_bf16)

    # ---- window index builder ----
    def windows(tile_blocks_base, ntiles, pos, pos_base_blockoff):
        """For tiles t=0..ntiles-1 over slots [c[B+t], c[B+t+1]),
        B = tile_blocks_base: per group windows, int16 local vs 1536*t.

        pos: position array indexed by slot; pos_base_blockoff: x-block index
        whose positions correspond to pos block 0 (4 for h1 table).
        Returns idx [ntiles, KI] int64 and (slots list per tile)."""
        idx = np.zeros((ntiles, KI), np.int64)
        tile_rows = []
        for t in range(ntiles):
            B = tile_blocks_base + t
            lo, hi = c[B], c[B + 1]
            base = 1536 * (B - pos_base_blockoff - 4)
            zero_w = base + 1533
            rows_here = np.full(SL, -1, np.int64)
            for gi, (dx, dy) in enumerate(GROUPS):
                km = K_OF[(dx, dy, -1)]
                k0 = K_OF[(dx, dy, 0)]
                kp = K_OF[(dx, dy, 1)]
                for u in range(SL):
                    j = lo + u
                    w = zero_w
                    if j < hi:
                        r = x0 + j
                        if 0 <= r < N:
                            rows_here[u] = r
                            j0 = nidx[k0, r]
                            if j0 < N:
                                w = pos[row_slot(j0)] - 1
                            else:
                                jm = nidx[km, r]
                                jp = nidx[kp, r]
                                if jm < N:
                                    w = pos[row_slot(jm)]
                                elif jp < N:
                                    w = pos[row_slot(jp)] - 2
                    lw = w - base
                    assert 0 <= lw <= WIN - 3, (core, t, gi, u, lw)
                    idx[t, gi * SL + u] = lw
            tile_rows.append(rows_here)
        return idx, tile_rows

    # conv1: tiles over x-blocks 4..61, windows into x table (x-block b sits
    # at position 1536b; base must be 1536*t = 1536*(B-4))
    idx1, rows1 = windows(4, NT1, pos_x, 0)
    # conv2: tiles over x-blocks 8..57 (= conv1 tiles 4..53), windows into h1
    # table where h1 block 0 = x-block 4 (so position of block B is
    # 1536*(B-4)); need base = 1536*t = 1536*(B-8) -> pos_base_blockoff = 4.
    idx2, rows2 = windows(8, NT2, pos_h, 4)

    # ---- scatter positions for conv1 tiles (local to slice 1536t) ----
    scpos = np.full((NT1, SL), DUMPL, np.int64)
    for t in range(NT1):
        rh = rows1[t]
        for u in range(SL):
            r = rh[u]
            if r >= 0:
                p = pos_h[row_slot(r)]
                assert p >= 0
                lp = p - 1536 * t
                assert 0 <= lp < 32000, (core, t, u, lp)
                scpos[t, u] = lp
    scw = _wrap_scatter(scpos)

    # ---- output mapping ----
    outmap = []  # (global_row, t2*SL + u)
    e = s + PER
    for t in range(NT2):
        rh = rows2[t]
        for u in range(SL):
            r = rh[u]
            if s <= r < e:
                outmap.append((r, t * SL + u))
    assert len(outmap) == PER, (core, len(outmap))

    return {
        "x_tab": x_tab,
        "i1": _wrap_idx(idx1),
        "i2": _wrap_idx(idx2),
        "sc": scw,
    }, outmap


def _prep_shared(W1, b1, W2, b2, W3, b3, W4, b4):
    W1p = np.zeros((K, C, C), np.float32)
    W1p[:, :IN_CH, :] = W1
    w1d = np.ascontiguousarray(
        W1p.transpose(1, 0, 2).reshape(C, K * C)).astype(_bf16)
    w2d = np.ascontiguousarray(
        W2.transpose(1, 0, 2).reshape(C, K * C)).astype(_bf16)
    w3d = np.ascontiguousarray(W3).astype(_bf16)
    w4d = np.zeros((C, 16), _bf16)
    w4d[:, 0] = W4[:, 0].astype(_bf16)
    return {
        "w1": w1d, "w2": w2d, "w3": w3d, "w4": w4d,
        "b1": np.ascontiguousarray(b1.reshape(C, 1), dtype=np.float32),
        "b2": np.ascontiguousarray(b2.reshape(C, 1), dtype=np.float32),
        "b3": np.ascontiguousarray(b3.reshape(C, 1), dtype=np.float32),
        "b4": np.ascontiguousarray(b4.reshape(1, 1), dtype=np.float32),
    }


_PREP_CACHE = {}


def _prep_in_maps(inputs):
    key = id(inputs.get("neighbor_idx"))
    nidx = np.asarray(inputs["neighbor_idx"]).astype(np.int32)
    features = np.asarray(inputs["features"], np.float32)
    t = np.asarray(inputs["t"])
    x_full = np.concatenate([features, _sinusoidal(t)], -1)
    shared_w = _prep_shared(
        np.asarray(inputs["W1"], np.float32), np.asarray(inputs["b1"], np.float32),
        np.asarray(inputs["W2"], np.float32), np.asarray(inputs["b2"], np.float32),
        np.asarray(inputs["W3"], np.float32), np.asarray(inputs["b3"], np.float32),
        np.asarray(inputs["W4"], np.float32), np.asarray(inputs["b4"], np.float32),
    )
    cont, shp = _global_flags(nidx)
    in_maps, outmaps = [], []
    for core in range(NCORES):
        m, om = _build_core(core, nidx, cont, shp, x_full)
        m.update(shared_w)
        in_maps.append(m)
        outmaps.append(om)
    return in_maps, outmaps


# ------------------------------------------------------------------ execution
def _run_pjrt(nc, in_maps, reps=0):
    import time as _time
    import jax
    from jax.sharding import Mesh, NamedSharding, PartitionSpec
    from jax.experimental.shard_map import shard_map
    import concourse.mybir as mybir
    from concourse import bass2jax

    bass2jax.install_neuronx_cc_hook()

    n_cores = len(in_maps)
    partition_name = (
        nc.partition_id_tensor.name if nc.partition_id_tensor else None
    )
    in_names, out_names, out_avals, zero_outs = [], [], [], []
    for alloc in nc.m.functions[0].allocations:
        if not isinstance(alloc, mybir.MemoryLocationSet):
            continue
        name = alloc.memorylocations[0].name
        if alloc.kind == "ExternalInput":
            if name != partition_name:
                in_names.append(name)
        elif alloc.kind == "ExternalOutput":
            shape = tuple(alloc.tensor_shape)
            dtype = mybir.dt.np(alloc.dtype)
            out_names.append(name)
            out_avals.append(jax.core.ShapedArray(shape, dtype))
            zero_outs.append(np.zeros(shape, dtype))
    n_params = len(in_names)
    n_outs = len(out_names)
    all_names = in_names + out_names
    if partition_name is not None:
        all_names = all_names + [partition_name]
    donate = tuple(range(n_params, n_params + n_outs))

    def _body(*args):
        operands = list(args)
        if partition_name is not None:
            operands.append(bass2jax.partition_id_tensor())
        outs = bass2jax._bass_exec_p.bind(
            *operands,
            out_avals=tuple(out_avals),
            in_names=tuple(all_names),
            out_names=tuple(out_names),
            lowering_input_output_aliases=(),
            sim_require_finite=True,
            sim_require_nnan=True,
            nc=nc,
        )
        return tuple(outs)

    devices = jax.devices()[:n_cores]
    mesh = Mesh(np.asarray(devices), ("core",))
    spec = PartitionSpec("core")
    sharded = jax.jit(
        shard_map(_body, mesh=mesh, in_specs=(spec,) * (n_params + n_outs),
                  out_specs=(spec,) * n_outs, check_rep=False),
        donate_argnums=donate,
        keep_unused=True,
    )
    concat_in = [
        np.concatenate([np.asarray(m[name]) for m in in_maps], axis=0)
        for name in in_names
    ]
    sh = NamedSharding(mesh, spec)
    inp_dev = [jax.device_put(a, sh) for a in concat_in]

    def _zeros():
        return [np.zeros((n_cores * z.shape[0], *z.shape[1:]), z.dtype)
                for z in zero_outs]

    out_arrs = sharded(*inp_dev, *_zeros())
    jax.block_until_ready(out_arrs)
    results = [
        {name: np.asarray(out_arrs[i]).reshape(n_cores, *out_avals[i].shape)[c]
         for i, name in enumerate(out_names)}
        for c in range(n_cores)
    ]

    times = []
    for _ in range(reps):
        zs = _zeros()
        t0 = _time.perf_counter()
        o = sharded(*inp_dev, *zs)
        jax.block_until_ready(o)
        times.append(_time.perf_counter() - t0)
    return results, times


def _assemble(results, outmaps):
    out = np.empty((N, 1), np.float32)
    for core in range(NCORES):
        r = results[core]["out"]
        for row, slot in outmaps[core]:
            out[row, 0] = r[slot]
    return out


def kernel(**inputs) -> np.ndarray:
    in_maps, outmaps = _prep_in_maps(inputs)
    nc = _get_nc()
    results, _ = _run_pjrt(nc, in_maps, reps=0)
    return _assemble(results, outmaps)


def bench(inputs, loop_reps=(1, 26), wall_reps=8):
    in_maps, outmaps = _prep_in_maps(inputs)
    walls = {}
    outs = {}
    for R in loop_reps:
        nc = _build_program(bench_reps=R)
        results, times = _run_pjrt(nc, in_maps, reps=wall_reps)
        walls[R] = min(times)
        outs[R] = _assemble(results, outmaps)
    R1, R2 = loop_reps
    per_iter = (walls[R2] - walls[R1]) / (R2 - R1)
    return per_iter, walls, outs
